# revision 7
# baseline (speedup 1.0000x reference)
"""Trainium2 Bass kernel for nn_Net_16999480558201 (gnn_message_passing).

Model (reference):
    feats = [x_graph | x_m[m_ids] | x_job[job_idx]]          # [N, 4H]
    h  = relu(feats @ W0 + b0); h = relu(h @ W1 + b1)
    s  = (h @ W2 + b2)[:, 0]                                  # [N]
    -> (argmax(s), softmax(s)[idx], log_softmax(s)[idx], entropy)

Kernel strategy (8 NeuronCores, data-parallel over N):
  * b2 and the uniform x_graph@W0[:2H] column-shift cancel in softmax
    outputs except through the relu, so x_graph contributes only via the
    constant c = x_graph @ W0[:2H] + b0 (computed on device).
  * x_m / x_job are cast to fp16 and stored in SBUF in a blocked layout;
    per-candidate rows are fetched with the SWDGE dma_gather (SBUF source,
    transpose=True) which lands them directly in [feature, row-tile] layout.
  * Per 512-row tile: 2 accumulating matmuls (W0 halves) + bias/relu,
    W1 matmul + bias/relu, then 4 matmuls with lhsT = h1 column-slices and
    rhs = W2 produce scores in the partition dimension of one PSUM bank.
  * Per-core softmax partials (max, sum(exp), sum(exp*s), argmax) are
    reduced on device; the 8x4 scalars are combined on the host.

Each core runs the same program on its 25000-candidate shard.
"""
import sys

if "/opt/trn_rl_repo" not in sys.path:
    sys.path.insert(0, "/opt/trn_rl_repo")

import numpy as np

H = 128
N = 200000
M = 1000
J = 5000
NCORES = 8
PER = N // NCORES            # 25000 rows per core
TILES = 49
T = 512                      # rows per tile
NPAD = TILES * T             # 25088
SCOLS = TILES * 4            # 196 score columns ([128, 196] layout)
RPP_M = 8                    # x_m rows per partition (blocked layout)
RPP_J = 40                   # x_job rows per partition
# gather chunking: SWDGE desc ring allows num_idxs/16+2 <= 64 slots/engine
# per instruction -> max 896 idxs per dma_gather.
CHUNK = 896                  # idxs per gather (= 7 cols of 128 rows)
NCHUNK = NPAD // CHUNK       # 28 chunks per table
HALF = CHUNK // 2            # 448-col matmul halves
NEG_BIG = -1.0e30
IOTA_BASE = 32768
BUILD_STAGE = "full"  # "gather" | "mlp" | "full" (bisect aid)            # idx encoded as IOTA_BASE - row

_CACHE = {}


def _build():
    import concourse.bacc as bacc
    import concourse.bass_isa as bass_isa
    import concourse.mybir as mybir
    import concourse.tile as tile
    from contextlib import ExitStack

    FP16 = mybir.dt.float16
    F32 = mybir.dt.float32
    I16 = mybir.dt.int16
    I32 = mybir.dt.int32
    AF = mybir.ActivationFunctionType
    ALU = mybir.AluOpType
    AX = mybir.AxisListType

    nc = bacc.Bacc("TRN2", target_bir_lowering=False, debug=False)

    xm_d = nc.dram_tensor("xm", [125, RPP_M * H], F32, kind="ExternalInput")
    xj_d = nc.dram_tensor("xj", [125, RPP_J * H], F32, kind="ExternalInput")
    w0a_d = nc.dram_tensor("w0a", [H, H], F32, kind="ExternalInput")
    w0b_d = nc.dram_tensor("w0b", [H, H], F32, kind="ExternalInput")
    w0g_d = nc.dram_tensor("w0g", [2 * H, H], F32, kind="ExternalInput")
    w1_d = nc.dram_tensor("w1", [H, H], F32, kind="ExternalInput")
    w2_d = nc.dram_tensor("w2", [H, 1], F32, kind="ExternalInput")
    b0_d = nc.dram_tensor("b0", [H, 1], F32, kind="ExternalInput")
    b1_d = nc.dram_tensor("b1", [H, 1], F32, kind="ExternalInput")
    xgt_d = nc.dram_tensor("xgt", [H, 2], F32, kind="ExternalInput")
    idm_d = nc.dram_tensor("idm", [128, NPAD // 16], I16, kind="ExternalInput")
    idj_d = nc.dram_tensor("idj", [128, NPAD // 16], I16, kind="ExternalInput")
    out_d = nc.dram_tensor("out", [1, 4], F32, kind="ExternalOutput")

    def _emit(tc, ctx):
        cpool = ctx.enter_context(tc.tile_pool(name="consts", bufs=1))
        gmpool = ctx.enter_context(tc.tile_pool(name="gm", bufs=2))
        gjpool = ctx.enter_context(tc.tile_pool(name="gj", bufs=2))
        hpool = ctx.enter_context(tc.tile_pool(name="h", bufs=3))
        rpool = ctx.enter_context(tc.tile_pool(name="red", bufs=1))
        ps0 = ctx.enter_context(tc.tile_pool(name="ps0", bufs=2, space="PSUM"))
        ps1 = ctx.enter_context(tc.tile_pool(name="ps1", bufs=2, space="PSUM"))
        psc = ctx.enter_context(tc.tile_pool(name="psc", bufs=1, space="PSUM"))
        psm = ctx.enter_context(tc.tile_pool(name="psm", bufs=1, space="PSUM"))

        # ---- constant / table loads ----
        xm16 = cpool.tile([128, RPP_M * H], FP16)
        nc.vector.memset(xm16[:, :], 0.0)
        nc.gpsimd.dma_start(out=xm16[0:125, :], in_=xm_d[:, :])
        xj16 = cpool.tile([128, RPP_J * H], FP16)
        nc.vector.memset(xj16[:, :], 0.0)
        nc.gpsimd.dma_start(out=xj16[0:125, :], in_=xj_d[:, :])

        idm_sb = cpool.tile([128, NPAD // 16], I16)
        nc.sync.dma_start(out=idm_sb[:, :], in_=idm_d[:, :])
        idj_sb = cpool.tile([128, NPAD // 16], I16)
        nc.sync.dma_start(out=idj_sb[:, :], in_=idj_d[:, :])

        def load_cast(dram, shape):
            t32 = cpool.tile(shape, F32, tag=f"ld32_{dram.name}")
            nc.sync.dma_start(out=t32[:, :], in_=dram[:, :])
            t16 = cpool.tile(shape, FP16, tag=f"ld16_{dram.name}")
            nc.vector.tensor_copy(t16[:, :], t32[:, :])
            return t16

        w0a16 = load_cast(w0a_d, [H, H])
        w0b16 = load_cast(w0b_d, [H, H])
        w1_16 = load_cast(w1_d, [H, H])
        w2_16 = load_cast(w2_d, [H, 1])
        xgt16 = load_cast(xgt_d, [H, 2])
        w0g16 = cpool.tile([128, 2 * H], FP16)   # two chunks side by side
        w0g32 = cpool.tile([128, 2 * H], F32)
        nc.sync.dma_start(out=w0g32[:, 0:H], in_=w0g_d[0:128, :])
        nc.sync.dma_start(out=w0g32[:, H: 2 * H], in_=w0g_d[128: 2 * H, :])
        nc.vector.tensor_copy(w0g16[:, :], w0g32[:, :])

        b0_sb = cpool.tile([H, 1], F32)
        nc.sync.dma_start(out=b0_sb[:, :], in_=b0_d[:, :])
        b1_sb = cpool.tile([H, 1], F32)
        nc.sync.dma_start(out=b1_sb[:, :], in_=b1_d[:, :])

        # ---- c = x_graph @ W0[:2H] + b0  (as [128, 1] per-feature bias) ----
        c_ps = psm.tile([128, 1], F32)
        nc.tensor.matmul(c_ps[:, :], w0g16[:, 0:H], xgt16[:, 0:1],
                         start=True, stop=False)
        nc.tensor.matmul(c_ps[:, :], w0g16[:, H: 2 * H], xgt16[:, 1:2],
                         start=False, stop=True)
        c_sb = cpool.tile([128, 1], F32)
        nc.scalar.activation(c_sb[:, :], c_ps[:, :], AF.Identity, bias=b0_sb[:, :])

        # ---- scores PSUM bank, pre-filled with -BIG for padding ----
        psc_t = psc.tile([128, SCOLS], F32)
        nc.vector.memset(psc_t[:, :], NEG_BIG)

        # ---- main pipeline over gather chunks ----
        for ch in range(NCHUNK):
            c16 = ch * (CHUNK // 16)
            gm_t = gmpool.tile([128, 1, CHUNK], FP16, tag="gm")
            gj_t = gjpool.tile([128, 1, CHUNK], FP16, tag="gj")
            nc.gpsimd.dma_gather(
                gm_t[:, :, :], xm16[:, :], idm_sb[:, c16: c16 + CHUNK // 16],
                CHUNK, CHUNK, H, transpose=True,
                sbuf_tokens_per_rank=128, sbuf_free_dim_per_rank=2 * H,
            )
            nc.gpsimd.dma_gather(
                gj_t[:, :, :], xj16[:, :], idj_sb[:, c16: c16 + CHUNK // 16],
                CHUNK, CHUNK, H, transpose=True,
                sbuf_tokens_per_rank=128, sbuf_free_dim_per_rank=2 * H,
            )

            if BUILD_STAGE == "gather":
                gsum = rpool.tile([128, 1], F32, tag="gsum")
                nc.vector.tensor_reduce(gsum[:, :], gm_t[:, 0, :],
                                        axis=AX.X, op=ALU.add)
                nc.vector.tensor_reduce(gsum[:, :], gj_t[:, 0, :],
                                        axis=AX.X, op=ALU.add)
                continue

            h1_t = hpool.tile([128, CHUNK], FP16, tag="h1")
            for hf in range(2):
                o = hf * HALF
                p0 = ps0.tile([128, HALF], F32, tag="p0")
                nc.tensor.matmul(p0[:, :], w0a16[:, :],
                                 gm_t[:, 0, o: o + HALF], start=True, stop=False)
                nc.tensor.matmul(p0[:, :], w0b16[:, :],
                                 gj_t[:, 0, o: o + HALF], start=False, stop=True)
                h0 = hpool.tile([128, HALF], FP16, tag="h0")
                nc.vector.tensor_scalar(h0[:, :], p0[:, :], c_sb[:, :], 0.0,
                                        op0=ALU.add, op1=ALU.max)
                p1 = ps1.tile([128, HALF], F32, tag="p1")
                nc.tensor.matmul(p1[:, :], w1_16[:, :], h0[:, :],
                                 start=True, stop=True)
                nc.scalar.activation(h1_t[:, o: o + HALF], p1[:, :], AF.Relu,
                                     bias=b1_sb[:, :])
            for c in range(7):
                col = ch * 7 + c
                row0 = col * 128
                nrows = min(128, PER - row0)
                if nrows <= 0:
                    break
                nc.tensor.matmul(
                    psc_t[0:nrows, col: col + 1],
                    h1_t[:, c * 128: c * 128 + nrows],
                    w2_16[:, :],
                    start=True, stop=True,
                )

        if BUILD_STAGE == "gather":
            out_sb = rpool.tile([1, 4], F32)
            nc.vector.memset(out_sb[:, :], 0.0)
            nc.vector.tensor_copy(out_sb[:, 0:1], gsum[0:1, 0:1])
            nc.sync.dma_start(out=out_d[:, :], in_=out_sb[:, :])
            return

        # ---- on-device softmax partials over scores [128, SCOLS] ----
        sc_sb = rpool.tile([128, SCOLS], F32)
        nc.scalar.activation(sc_sb[:, :], psc_t[:, :], AF.Copy)

        if BUILD_STAGE == "mlp":
            out_sb = rpool.tile([1, 4], F32)
            nc.vector.tensor_copy(out_sb[:, :], sc_sb[0:1, 0:4])
            nc.sync.dma_start(out=out_d[:, :], in_=out_sb[:, :])
            return

        rmax = rpool.tile([128, 1], F32)
        nc.vector.tensor_reduce(rmax[:, :], sc_sb[:, :], axis=AX.X, op=ALU.max)
        mxb = rpool.tile([128, 1], F32)
        nc.gpsimd.partition_all_reduce(mxb[:, :], rmax[:, :], 128,
                                       bass_isa.ReduceOp.max)
        negmx = rpool.tile([128, 1], F32)
        nc.vector.tensor_scalar(negmx[:, :], mxb[:, :], -1.0, None, op0=ALU.mult)

        expd = rpool.tile([128, SCOLS], F32)
        zrow = rpool.tile([128, 1], F32)
        nc.scalar.activation(expd[:, :], sc_sb[:, :], AF.Exp,
                             bias=negmx[:, :], accum_out=zrow[:, :])
        sxe = rpool.tile([128, SCOLS], F32)
        nc.vector.tensor_tensor(sxe[:, :], expd[:, :], sc_sb[:, :], op=ALU.mult)
        srow = rpool.tile([128, 1], F32)
        nc.vector.tensor_reduce(srow[:, :], sxe[:, :], axis=AX.X, op=ALU.add)
        zsum = rpool.tile([128, 1], F32)
        nc.gpsimd.partition_all_reduce(zsum[:, :], zrow[:, :], 128,
                                       bass_isa.ReduceOp.add)
        ssum = rpool.tile([128, 1], F32)
        nc.gpsimd.partition_all_reduce(ssum[:, :], srow[:, :], 128,
                                       bass_isa.ReduceOp.add)

        eqm = rpool.tile([128, SCOLS], F32)
        nc.vector.tensor_scalar(eqm[:, :], sc_sb[:, :], mxb[:, :], None,
                                op0=ALU.is_equal)
        iota32 = rpool.tile([128, SCOLS], I32)
        nc.gpsimd.iota(iota32[:, :], pattern=[[128, SCOLS]], base=0,
                       channel_multiplier=1)
        iotaf0 = rpool.tile([128, SCOLS], F32)
        nc.vector.tensor_copy(iotaf0[:, :], iota32[:, :])
        iotaf = rpool.tile([128, SCOLS], F32)
        nc.vector.tensor_scalar(iotaf[:, :], iotaf0[:, :], -1.0, float(IOTA_BASE),
                                op0=ALU.mult, op1=ALU.add)
        cand = rpool.tile([128, SCOLS], F32)
        nc.vector.tensor_tensor(cand[:, :], eqm[:, :], iotaf[:, :], op=ALU.mult)
        crow = rpool.tile([128, 1], F32)
        nc.vector.tensor_reduce(crow[:, :], cand[:, :], axis=AX.X, op=ALU.max)
        idxn = rpool.tile([128, 1], F32)
        nc.gpsimd.partition_all_reduce(idxn[:, :], crow[:, :], 128,
                                       bass_isa.ReduceOp.max)

        out_sb = rpool.tile([1, 4], F32)
        nc.vector.tensor_copy(out_sb[:, 0:1], mxb[0:1, :])
        nc.vector.tensor_copy(out_sb[:, 1:2], zsum[0:1, :])
        nc.vector.tensor_copy(out_sb[:, 2:3], ssum[0:1, :])
        nc.vector.tensor_copy(out_sb[:, 3:4], idxn[0:1, :])
        nc.sync.dma_start(out=out_d[:, :], in_=out_sb[:, :])

    with tile.TileContext(nc) as tc, ExitStack() as ctx:
        _emit(tc, ctx)

    nc.compile()
    return nc


def _get_nc():
    if "nc" not in _CACHE:
        _CACHE["nc"] = _build()
    return _CACHE["nc"]


def _wrap_idx(sigma):
    """[NPAD] int -> [128, NPAD//16] int16, idx i at (i%16, i//16), x8 replicated."""
    w16 = sigma.astype(np.int16).reshape(NPAD // 16, 16).T  # [16, NPAD//16]
    return np.tile(w16, (8, 1))


def _prep_in_maps(x_graph, x_m, x_job, m_ids, job_idx, W0, b0, b1):
    x_m = np.asarray(x_m, np.float32)
    x_job = np.asarray(x_job, np.float32)
    W0 = np.asarray(W0, np.float32)
    xm_blk = x_m.reshape(125, RPP_M * H)
    xj_blk = x_job.reshape(125, RPP_J * H)
    shared = {
        "xm": xm_blk,
        "xj": xj_blk,
        "w0g": np.ascontiguousarray(W0[0: 2 * H]),
        "w0a": np.ascontiguousarray(W0[2 * H: 3 * H]),
        "w0b": np.ascontiguousarray(W0[3 * H: 4 * H]),
        "w1": None,  # filled by caller
        "w2": None,
        "b0": np.asarray(b0, np.float32).reshape(H, 1),
        "b1": np.asarray(b1, np.float32).reshape(H, 1),
        "xgt": np.asarray(x_graph, np.float32).reshape(2, H).T.copy(),
    }
    m_ids = np.asarray(m_ids).astype(np.int64)
    job_idx = np.asarray(job_idx).astype(np.int64)
    in_maps = []
    for k in range(NCORES):
        mk = m_ids[k * PER: (k + 1) * PER]
        jk = job_idx[k * PER: (k + 1) * PER]
        mk = np.concatenate([mk, np.zeros(NPAD - PER, np.int64)])
        jk = np.concatenate([jk, np.zeros(NPAD - PER, np.int64)])
        sig_m = (mk % RPP_M) * 128 + mk // RPP_M
        sig_j = (jk % RPP_J) * 128 + jk // RPP_J
        in_maps.append({
            **{n: v for n, v in shared.items() if v is not None},
            "idm": _wrap_idx(sig_m),
            "idj": _wrap_idx(sig_j),
        })
    return in_maps


def kernel(x_graph, x_m, x_job, m_ids, job_idx, W0, b0, W1, b1, W2, b2,
           _trace=False):
    from concourse.bass_utils import run_bass_kernel_spmd

    nc = _get_nc()
    in_maps = _prep_in_maps(x_graph, x_m, x_job, m_ids, job_idx, W0, b0, b1)
    w1 = np.asarray(W1, np.float32)
    w2 = np.asarray(W2, np.float32).reshape(H, 1)
    for im in in_maps:
        im["w1"] = w1
        im["w2"] = w2

    res = run_bass_kernel_spmd(nc, in_maps, list(range(NCORES)), trace=_trace)
    outs = np.stack([res.results[k]["out"][0] for k in range(NCORES)])
    if _trace:
        _CACHE["last_result"] = res

    mx = outs[:, 0].astype(np.float64)
    Z = outs[:, 1].astype(np.float64)
    S = outs[:, 2].astype(np.float64)
    lidx = (IOTA_BASE - outs[:, 3]).astype(np.int64)

    gm = mx.max()
    kstar = int(np.argmax(mx))
    w = np.exp(mx - gm)
    Zg = float((Z * w).sum())
    Sg = float((S * w).sum())
    lse = gm + np.log(Zg)
    entropy = lse - Sg / Zg
    idx = kstar * PER + int(lidx[kstar])
    logp = float(gm - lse)
    prob = float(np.exp(logp))
    return (np.int32(idx), np.float32(prob), np.float32(logp),
            np.float32(entropy))
